# revision 32
# baseline (speedup 1.0000x reference)
"""Causal self-attention with RoPE on 8 Trainium2 NeuronCores.

Sharding: tensor-parallel over heads (2 heads/core) through QKV projection,
RoPE and attention; AllToAll reshards attention output from head-split to
token-split; out-projection is token-parallel with full out_w per core
(no reduction needed). Output: each core produces its 512-token slice.

fp8 DoubleRow matmuls: the QKV and output projections run on the PE in
fp8e4m3 DoubleRow mode (2 contraction tiles per pass at 0.5 cycles/row =
4x bf16 rate per pass). Operands are represented as residual pairs
(x ~= x0 + x1, W ~= W0 + W1, each plane fp8e4m3) and the product is
computed as W0x0 + W1x0 + W0x1 (12N cycles vs bf16's 16N), dropping only
the W1x1 term (~4e-4 relative). Pair splitting for x and weights happens
on the host; attention output is split on-device at softmax eviction and
shipped through the AllToAll as an fp8 pair (same bytes as bf16).

Layouts (per core, f = feature, t = token, d = contraction):
  x0T/x1T [D, NT]  input fp8 pair, d on partitions - rhs/lhsT for projections
  q/k  [128, NT]  per-head, head-dim on partitions ("qT"): proj out [f, t]
  v    [NT, 256]  token-major: proj out [t, f]
  sT   [j, r]     scores transposed: lhsT=kT-tile, rhs=qT-block
  pT   [j, r]     exp(scores*scale) bf16
  oT   [dv, r]    PV: lhsT=v-tile [j, dv], rhs=pT [j, r]
  denom[1, r]     ones-matmul over dacc (fp32 accumulated pT)
  out  [t, e]     out-proj: lhsT=attnT-tile [dv, t], rhs=out_wT [dv, e]
"""
import math
import numpy as np
import ml_dtypes

import concourse.bass as bass
import concourse.mybir as mybir
import concourse.tile as tile
from concourse import bacc
from concourse.bass_utils import run_bass_kernel_spmd

F32 = mybir.dt.float32
BF16 = mybir.dt.bfloat16
FP8 = mybir.dt.float8e4
AF = mybir.ActivationFunctionType
ALU = mybir.AluOpType
DR = mybir.MatmulPerfMode.DoubleRow

N_CORES = 8


def legalize_waits(nc, max_waits=1):
    """This walrus build only encodes one sync-wait per TPB instruction.
    Move extra waits emitted by Tile onto same-engine NoOps inserted
    immediately before the instruction."""
    n_split = 0
    for fn in nc.m.functions:
        for bb in fn.blocks:
            new_insts = []
            for inst in bb.instructions:
                si = getattr(inst, "sync_info", None)
                waits = list(si.on_wait) if si is not None and si.on_wait else []
                if len(waits) > max_waits and type(inst).__name__ != "InstNoOp":
                    extra, keep = waits[:-max_waits], waits[-max_waits:]
                    for k, w in enumerate(extra):
                        nop = mybir.InstNoOp(
                            name=f"{inst.name}_waitnop{k}",
                            engine=inst.engine,
                            ins=[],
                            outs=[],
                            sync_info=mybir.SyncInfo(on_wait=[w], on_update=[]),
                        )
                        nc.register_instruction(nop)
                        new_insts.append(nop)
                    inst.sync_info = mybir.SyncInfo(
                        on_wait=keep, on_update=list(si.on_update)
                    )
                    n_split += 1
                new_insts.append(inst)
            bb.instructions = new_insts
    return n_split


def build_nc(B=2, T=2048, D=2048, H=16, fake_cc=False, n_loop=1, dummy_io=False):
    HD = D // H                  # 128, head dim
    NT = B * T                   # total tokens
    HPC = H // N_CORES           # heads per core (2)
    DC = HPC * HD                # head channels per core (256)
    KT = D // 128                # contraction tiles for projections (16)
    NB = NT // 512               # 512-token blocks overall (8)
    RB = T // 512                # 512-token blocks per batch element (4)
    S = NT // N_CORES            # AllToAll shard = tokens per core (512)
    EB = D // 512                # 512-wide out-feature blocks (4)
    SCALE = 1.0 / math.sqrt(HD)

    nc = bacc.Bacc("TRN2", target_bir_lowering=False, debug=False, num_devices=N_CORES)
    # dummy_io: declare data tensors as internal DRAM (uninitialized) so the
    # timing NEFF has no big inputs to ship through the axon tunnel.
    ik = {"kind": "ExternalInput"} if not dummy_io else {}
    x0T_e = nc.dram_tensor("x0T", [D, NT], FP8, **ik)
    x1T_e = nc.dram_tensor("x1T", [D, NT], FP8, **ik)
    wqk0_e = nc.dram_tensor("wqk0", [D, 4 * HD], FP8, **ik)
    wqk1_e = nc.dram_tensor("wqk1", [D, 4 * HD], FP8, **ik)
    bqk_e = nc.dram_tensor("bqk", [128, 4], F32, **ik)
    wv0_e = nc.dram_tensor("wv0", [D, DC], FP8, **ik)
    wv1_e = nc.dram_tensor("wv1", [D, DC], FP8, **ik)
    bv_e = nc.dram_tensor("bv", [128, DC], F32, **ik)
    cos_e = nc.dram_tensor("cosT", [HD, NT], BF16, **ik)
    sin_e = nc.dram_tensor("sinT", [HD, NT], BF16, **ik)
    masks_e = nc.dram_tensor("masks", [128, 4, 512], BF16, **ik)
    ow0_e = nc.dram_tensor("ow0", [D, D], FP8, **ik)
    ow1_e = nc.dram_tensor("ow1", [D, D], FP8, **ik)
    ob_e = nc.dram_tensor("ob", [128, D], F32, **ik)
    out_e = nc.dram_tensor("out", [S, D], BF16, kind="ExternalOutput")

    with tile.TileContext(nc) as tc:
      for _it in range(n_loop):
        with tc.tile_pool(name=f"persist{_it}", bufs=1) as pp, \
             tc.tile_pool(name=f"dram{_it}", bufs=1, space="DRAM") as dp:
            # ---- persistent tiles ----
            qk = [pp.tile([128, NT], BF16, tag=f"qk{m}", name=f"qk{m}") for m in range(4)]
            v_sb = pp.tile([128, NT // 128, DC], BF16, tag="v", name="v")
            masks = pp.tile([128, 4, 512], BF16, tag="masks", name="masks")
            nc.gpsimd.dma_start(masks[:], masks_e[:])
            bqk = pp.tile([128, 4], F32, tag="bqk", name="bqk")
            nc.gpsimd.dma_start(bqk[:], bqk_e[:])
            # biases pre-broadcast on host; loaded via Pool software DMA so
            # they never occupy the shared hardware DGE path
            bvB = pp.tile([128, DC], F32, tag="bvB", name="bvB")
            nc.gpsimd.dma_start(bvB[:], bv_e[:])
            obB = pp.tile([128, D], F32, tag="obB", name="obB")
            nc.gpsimd.dma_start(obB[:], ob_e[:])
            # ones-vector carries 1/16 so the replicated denominator is
            # denom/16 and recipB = 16/denom: the o pair then holds 16*o,
            # centering it in e4m3's exponent range. The out-projection
            # eviction descales by 1/(16*64).
            ones = pp.tile([128, 128], BF16, tag="ones", name="ones")
            nc.vector.memset(ones[:], 1.0 / 16)

            # o shipped as fp8 residual pair: plane 0 = fp8(o), plane 1 = o - plane0
            Zs = [dp.tile([N_CORES, 2, HD, S], FP8, tag=f"Z{i}", name=f"Z{i}")
                  for i in range(HPC)]
            ZGs = [dp.tile([N_CORES, 2, HD, S], FP8, tag=f"ZG{i}", name=f"ZG{i}")
                   for i in range(HPC)]

            # pass-1 out-proj weights live in a pool opened before the
            # phase-1 pools: no SBUF anti-dependency, so their ACT-ring loads
            # stream during phase-1 compute.
            owpool1 = tc.tile_pool(name=f"owp1_{_it}", bufs=1)
            owpre1 = owpool1.__enter__()
            ow_pre1 = [[owpre1.tile([128, KT // 2, 512], FP8,
                                    tag=f"owp0_{e}_{i}", name=f"owp0_{e}_{i}")
                        for i in range(2)] for e in range(EB)]

            # ---- phase 1: QKV projections (fp8 DoubleRow) + RoPE ----
            with tc.tile_pool(name=f"p1w{_it}", bufs=1) as wp, \
                 tc.tile_pool(name=f"p1x{_it}", bufs=3) as xp, \
                 tc.tile_pool(name=f"p1t{_it}", bufs=3) as tp, \
                 tc.tile_pool(name=f"p1ps{_it}", bufs=6, space="PSUM") as ps:

                # ACT HW ring order: qk weights gate the first matmuls, then
                # cos/sin for the first RoPE, then v weights (v matmuls start
                # ~15us in).
                wqk_sb = [wp.tile([128, KT, 4 * HD], FP8, tag=f"wqk{i}", name=f"wqk{i}")
                          for i in range(2)]
                for i, we in enumerate((wqk0_e, wqk1_e)):
                    wr = we.rearrange("(kt p) f -> p kt f", p=128)
                    nc.scalar.dma_start(wqk_sb[i][:, :KT // 2, :], wr[:, :KT // 2, :])
                    nc.scalar.dma_start(wqk_sb[i][:, KT // 2:, :], wr[:, KT // 2:, :])
                wv_sb = [wp.tile([128, KT, DC], FP8, tag=f"wv{i}", name=f"wv{i}")
                         for i in range(2)]
                for i, we in enumerate((wv0_e, wv1_e)):
                    nc.scalar.dma_start(wv_sb[i][:], we.rearrange("(kt p) f -> p kt f", p=128))
                cos_sb = wp.tile([128, NT], BF16, tag="cos", name="cos")
                nc.scalar.dma_start(cos_sb[:], cos_e[:])
                sin_sb = wp.tile([128, NT], BF16, tag="sin", name="sin")
                nc.scalar.dma_start(sin_sb[:], sin_e[:])
                for e in range(EB):
                    for i, owe in enumerate((ow0_e, ow1_e)):
                        owr = owe.rearrange("(kt p) f -> p kt f", p=128)[:, 0::2, bass.ts(e, 512)]
                        nc.gpsimd.dma_start(ow_pre1[e][i][:, :KT // 4, :], owr[:, :KT // 4, :])
                        nc.gpsimd.dma_start(ow_pre1[e][i][:, KT // 4:, :], owr[:, KT // 4:, :])
                for blk in range(NB):
                    tsl = bass.ts(blk, 512)
                    x0r = x0T_e.rearrange("(kt p) t -> p kt t", p=128)
                    x1r = x1T_e.rearrange("(kt p) t -> p kt t", p=128)
                    xb = [xp.tile([128, KT, 512], FP8, tag=f"x{i}b", name=f"x{i}b")
                          for i in range(2)]
                    for i, xr in enumerate((x0r, x1r)):
                        for h in range(4):
                            ksl = bass.ds(h * (KT // 4), KT // 4)
                            nc.sync.dma_start(xb[i][:, ksl, :], xr[:, ksl, tsl])
                    # (lhsT plane source, rhs plane source) per partial product:
                    # W0x0 + W1x0 + W0x1
                    qk_prods = ((wqk_sb[0], xb[0]), (wqk_sb[1], xb[0]), (wqk_sb[0], xb[1]))
                    # q/k projections -> [f, t], f-tile m: q0 q1 k0 k1
                    for m in range(4):
                        psqk = ps.tile([128, 512], F32, tag="ps", name="ps")
                        for nh in range(2):
                            for pi, (wt, xt) in enumerate(qk_prods):
                                for t in range(KT // 2):
                                    nc.tensor.matmul(
                                        psqk[:, bass.ts(nh, 256)],
                                        wt[:, 2 * t:2 * t + 2, bass.ts(m, 128)],
                                        xt[:, 2 * t:2 * t + 2, bass.ds(nh * 256, 256)],
                                        start=(pi == 0 and t == 0),
                                        stop=(pi == 2 and t == KT // 2 - 1),
                                        perf_mode=DR,
                                    )
                        # evict with bias on ACT
                        nc.scalar.activation(
                            qk[m][:, tsl], psqk[:], AF.Identity,
                            bias=bqk[:, m:m + 1], scale=1.0 / 64,
                        )
                    # v projection -> [t, f]: x tile is lhsT here
                    v_prods = ((xb[0], wv_sb[0]), (xb[0], wv_sb[1]), (xb[1], wv_sb[0]))
                    for tt in range(4):
                        psv = ps.tile([128, 512], F32, tag="ps", name="ps")
                        for pi, (xt, wt) in enumerate(v_prods):
                            for t in range(KT // 2):
                                nc.tensor.matmul(
                                    psv[:, :DC],
                                    xt[:, 2 * t:2 * t + 2, bass.ts(tt, 128)],
                                    wt[:, 2 * t:2 * t + 2, :],
                                    start=(pi == 0 and t == 0),
                                    stop=(pi == 2 and t == KT // 2 - 1),
                                    perf_mode=DR,
                                )
                        nc.vector.scalar_tensor_tensor(
                            v_sb[:, blk * 4 + tt, :], psv[:, :DC], 1.0 / 64, bvB[:],
                            ALU.mult, ALU.add)
                    # RoPE in place: qk = qk*cos + swap(qk)*s2, where s2 = sin
                    # with first half negated (host-prepared) and swap
                    # exchanges partition halves (engines cannot read across
                    # partitions -> SBUF->SBUF DMA). Emitted after the v-adds
                    # so the in-order DVE queue isn't blocked on cos/sin
                    # arrival at startup.
                    for m in range(4):
                        qm = qk[m][:, tsl]
                        qsw = tp.tile([128, 512], BF16, tag="qsw", name="qsw")
                        nc.sync.dma_start(qsw[0:64, :], qm[64:128, :])
                        nc.sync.dma_start(qsw[64:128, :], qm[0:64, :])
                        nc.vector.tensor_mul(qsw[:], qsw[:], sin_sb[:, tsl])
                        nc.vector.tensor_mul(qm, qm, cos_sb[:, tsl])
                        nc.vector.tensor_add(qm, qm, qsw[:])

            zg_sb = [[pp.tile([128, N_CORES, S], FP8, tag=f"zg{i}_{pl}", name=f"zg{i}_{pl}")
                      for pl in range(2)] for i in range(HPC)]

            # preload pass-2 out-proj weights (odd-head rows, fp8 pair): the
            # pool overlaps phase-1 SBUF, so the transfers start as phase-1
            # tiles retire -- well before phase 4. Keeping these and the zg
            # loads off the Pool ring leaves the collective dispatch instant.
            with tc.tile_pool(name=f"owpre{_it}", bufs=1) as owpre:
                ow_pre2 = [[owpre.tile([128, KT // 2, 512], FP8,
                                       tag=f"owp1_{e}_{i}", name=f"owp1_{e}_{i}")
                            for i in range(2)] for e in range(EB)]
                for e in range(EB):
                    for i, owe in enumerate((ow0_e, ow1_e)):
                        owr = owe.rearrange("(kt p) f -> p kt f", p=128)[:, 1::2, bass.ts(e, 512)]
                        nc.gpsimd.dma_start(ow_pre2[e][i][:, :KT // 4, :], owr[:, :KT // 4, :])
                        nc.gpsimd.dma_start(ow_pre2[e][i][:, KT // 4:, :], owr[:, KT // 4:, :])
                ow_pre = [ow_pre1, ow_pre2]

                # ---- phase 2: attention per (head, batch) ----
                # 1024-wide score chunks (2 key-tiles per exp) amortize ACT
                # overhead. The softmax denominator is accumulated on the PE
                # via a ones-vector matmul per chunk half (PSUM accumulation
                # across the whole row block). Software-pipelined: scores+exp
                # of chunk i+1 are issued before PV/denominator of chunk i, so
                # the PE never waits for the exp at row-block starts. hh-outer
                # so each head-half's AllToAll fires while the other computes.
                with tc.tile_pool(name=f"p2t{_it}", bufs=4) as tp2, \
                     tc.tile_pool(name=f"p2ps{_it}", bufs=2, space="PSUM") as ps2, \
                     tc.tile_pool(name=f"p2po{_it}", bufs=2, space="PSUM") as ps2o, \
                     tc.tile_pool(name=f"p2pd{_it}", bufs=2, space="PSUM") as ps2d:
                    # chunk list: per row block, 2*rb+1 full-width chunks
                    # (2 key tiles x 512 queries) then one half chunk: the
                    # final 2 key tiles are only valid for the last 256
                    # queries of the block (causal diagonal trim).
                    chunks = []
                    for hh in range(HPC):
                        for b in range(B):
                            for rb in range(RB):
                                nfull = 2 * rb + 1
                                for jc in range(nfull + 1):
                                    chunks.append((hh, b, rb, jc, nfull))

                    def emit_scores(c):
                        hh, b, rb, jc, nfull = c
                        half_chunk = jc == nfull
                        qoff, width = (256, 256) if half_chunk else (0, 512)
                        rsl = bass.ds(b * T + rb * 512 + qoff, width)
                        pss = ps2.tile([128, 1024], F32, tag="pss", name="pss")
                        for half in range(2):
                            jt = 2 * jc + half
                            nc.tensor.matmul(
                                pss[:, bass.ds(half * width, width)],
                                qk[2 + hh][:, bass.ds(b * T + jt * 128, 128)],
                                qk[hh][:, rsl],
                                start=True, stop=True,
                            )
                        W = 2 * width
                        pT = tp2.tile([128, 1024], BF16, tag="pT", name="pT")
                        nc.scalar.activation(pT[:, :W], pss[:, :W], AF.Exp, scale=SCALE)
                        if half_chunk:
                            pTv = pT[:, 0:W].rearrange("p (a b) -> p a b", a=2)
                            nc.vector.tensor_mul(pTv, pTv, masks[:, 0:2, 0:256])
                        elif jc == 2 * rb:
                            nc.vector.tensor_mul(
                                pT[:, :W], pT[:, :W],
                                masks[:, 0:2, :].rearrange("p a b -> p (a b)"),
                            )
                        return pT

                    rbs = {}

                    def emit_pv(c, pT):
                        hh, b, rb, jc, nfull = c
                        half_chunk = jc == nfull
                        qoff, width = (256, 256) if half_chunk else (0, 512)
                        if jc == 0:
                            rbs["pso"] = ps2o.tile([128, 512], F32, tag="pso", name="pso")
                            rbs["psd"] = ps2d.tile([128, 512], F32, tag="psd", name="psd")
                        pso, psd = rbs["pso"], rbs["psd"]
                        for half in range(2):
                            jt = 2 * jc + half
                            nc.tensor.matmul(
                                psd[:, bass.ds(qoff, width)], ones[:],
                                pT[:, bass.ds(half * width, width)],
                                start=(jt == 0), stop=(half_chunk and half == 1),
                                skip_group_check=True,
                            )
                            nc.tensor.matmul(
                                pso[:, bass.ds(qoff, width)],
                                v_sb[:, (b * T) // 128 + jt, bass.ts(hh, HD)],
                                pT[:, bass.ds(half * width, width)],
                                start=(jt == 0),
                                stop=(half_chunk and half == 1),
                                skip_group_check=True,
                            )
                        if not half_chunk:
                            return
                        # row block done: normalize, split to fp8 pair, ship
                        # (denominator already replicated across partitions)
                        recipB = tp2.tile([128, 512], F32, tag="recipB", name="recipB")
                        nc.vector.reciprocal(recipB[:], psd[:])
                        onrm = tp2.tile([128, 512], F32, tag="onrm", name="onrm")
                        nc.vector.tensor_mul(onrm[:], pso[:], recipB[:])
                        o0 = tp2.tile([128, 512], FP8, tag="o0", name="o0")
                        nc.scalar.activation(o0[:], onrm[:], AF.Copy)
                        o1 = tp2.tile([128, 512], FP8, tag="o1", name="o1")
                        nc.vector.tensor_sub(o1[:], onrm[:], o0[:])
                        g = b * RB + rb
                        nc.sync.dma_start(Zs[hh][g, 0, :, :], o0[:])
                        nc.sync.dma_start(Zs[hh][g, 1, :, :], o1[:])
                        if not (b == B - 1 and rb == RB - 1):
                            return
                        # head-half done: reshard while the next one computes
                        if fake_cc:
                            nc.sync.dma_start(ZGs[hh][:], Zs[hh][:])
                        else:
                            nc.gpsimd.collective_compute(
                                "AllToAll", ALU.bypass,
                                replica_groups=[list(range(N_CORES))],
                                ins=[Zs[hh][:]], outs=[ZGs[hh][:]],
                            )
                        # pull out-proj operands into SBUF via the ACT
                        # hardware ring: every other transfer on that ring was
                        # issued earlier, so the wait on the collective blocks
                        # nothing.
                        for pl in range(2):
                            nc.scalar.dma_start(
                                zg_sb[hh][pl][:],
                                ZGs[hh][:, pl].rearrange("c d s -> d c s"),
                            )

                    prev = None
                    for c in chunks:
                        pT = emit_scores(c)
                        if prev is not None:
                            emit_pv(*prev)
                        prev = (c, pT)
                    emit_pv(*prev)

                # ---- phase 4: out projection on own token slice (fp8 DR) ----
                # Two-pass contraction: all zg[0] (first AllToAll) partial sums
                # are computed and evicted before any zg[1] tile is touched, so
                # the second AllToAll and its SBUF load hide behind real matmul
                # work. Pass 1 contracts even heads (hh=0), pass 2 odd heads.
                with tc.tile_pool(name=f"p4z{_it}", bufs=1) as zp, \
                     tc.tile_pool(name=f"p4t{_it}", bufs=4) as tp4, \
                     tc.tile_pool(name=f"p4ps{_it}", bufs=4, space="PSUM") as ps4:
                    accbig = zp.tile([128, EB * (S // 128), 512], F32, tag="accbig", name="accbig")
                    for pas in range(2):
                        zg0, zg1 = zg_sb[pas]
                        for e in range(EB):
                            # rows of out_w for heads (2*zt + pas), fp8 pair
                            ow = ow_pre[pas][e]
                            prods = ((zg0, ow[0]), (zg0, ow[1]), (zg1, ow[0]))
                            for tt in range(S // 128):
                                pso4 = ps4.tile([128, 512], F32, tag="ps4", name="ps4")
                                for nh in range(2):
                                    for pi, (zt_src, owt) in enumerate(prods):
                                        for t in range(KT // 4):
                                            nc.tensor.matmul(
                                                pso4[:, bass.ts(nh, 256)],
                                                zt_src[:, 2 * t:2 * t + 2, bass.ts(tt, 128)],
                                                owt[:, 2 * t:2 * t + 2, bass.ds(nh * 256, 256)],
                                                start=(pi == 0 and t == 0),
                                                stop=(pi == 2 and t == KT // 4 - 1),
                                                perf_mode=DR,
                                            )
                                if pas == 0:
                                    nc.vector.scalar_tensor_tensor(
                                        accbig[:, e * (S // 128) + tt, :], pso4[:],
                                        1.0 / (16 * 64), obB[:, bass.ts(e, 512)],
                                        ALU.mult, ALU.add)
                                else:
                                    of = tp4.tile([128, 512], BF16, tag="of", name="of")
                                    nc.vector.scalar_tensor_tensor(
                                        of[:], pso4[:], 1.0 / (16 * 64),
                                        accbig[:, e * (S // 128) + tt, :],
                                        ALU.mult, ALU.add)
                                    nc.sync.dma_start(out_e[bass.ts(tt, 128), bass.ts(e, 512)], of[:])
            owpool1.__exit__(None, None, None)

    nc.compile()          # Bacc pass pipeline (library loads, nop fusion, regs)
    legalize_waits(nc)    # must run after all nop-fusion passes
    bass.Bass.finalize(nc)  # freeze without re-running Bacc compile
    return nc


def _fp8_pair(a, scale=1.0):
    """Split scale*a into an fp8e4m3 residual pair (a0 + a1 ~= scale*a).

    Weights (sigma ~ 1/sqrt(D) ~ 0.02) sit at the bottom of e4m3's exponent
    range, where the residual plane underflows into subnormals (min 2^-9);
    pre-scaling by 64 recovers full pair precision. The inverse scale is
    folded into the PSUM eviction."""
    f8 = ml_dtypes.float8_e4m3
    a = np.ascontiguousarray(a, dtype=np.float32) * np.float32(scale)
    a0 = a.astype(f8)
    a1 = (a - a0.astype(np.float32)).astype(f8)
    return a0, a1


def _prep_inputs(x, rope_cos, rope_sin, qkv_w, qkv_b, out_w, out_b, B, T, D, H):
    HD = D // H
    NT = B * T
    HPC = H // N_CORES
    bf = ml_dtypes.bfloat16

    x2 = np.ascontiguousarray(x.reshape(NT, D).T.astype(np.float32))     # [D, NT]
    x0, x1 = _fp8_pair(x2)
    cosT = np.ascontiguousarray(
        np.tile(rope_cos[0, 0].T, (1, B))).astype(bf)                   # [HD, NT]
    s2 = np.tile(rope_sin[0, 0].T, (1, B)).copy()
    s2[:HD // 2] *= -1.0
    sinT = np.ascontiguousarray(s2).astype(bf)
    ow0, ow1 = _fp8_pair(out_w.T, scale=64.0)                            # [D, D]
    obB = np.ascontiguousarray(
        np.broadcast_to(out_b.astype(np.float32), (128, D)))             # [128, D]

    c_grid = np.arange(512)[None, :]
    p_grid = np.arange(128)[:, None]
    masks = np.ascontiguousarray(np.stack(
        [(c_grid >= 128 * m + p_grid) for m in range(4)]
    ).transpose(1, 0, 2)).astype(bf)                                    # [128,4,512]

    in_maps = []
    for c in range(N_CORES):
        heads = [HPC * c + i for i in range(HPC)]
        q_rows = np.concatenate([qkv_w[h * HD:(h + 1) * HD] for h in heads])
        k_rows = np.concatenate([qkv_w[D + h * HD:D + (h + 1) * HD] for h in heads])
        v_rows = np.concatenate([qkv_w[2 * D + h * HD:2 * D + (h + 1) * HD] for h in heads])
        wqk0, wqk1 = _fp8_pair(np.concatenate([q_rows, k_rows]).T, scale=64.0)
        wv0, wv1 = _fp8_pair(v_rows.T, scale=64.0)
        bq = np.concatenate([qkv_b[h * HD:(h + 1) * HD] for h in heads])
        bk = np.concatenate([qkv_b[D + h * HD:D + (h + 1) * HD] for h in heads])
        bqk = np.ascontiguousarray(
            np.concatenate([bq, bk]).astype(np.float32).reshape(4, 128).T)
        bv = np.concatenate(
            [qkv_b[2 * D + h * HD:2 * D + (h + 1) * HD] for h in heads]
        ).astype(np.float32)
        bvB = np.ascontiguousarray(np.broadcast_to(bv, (128, len(bv))))
        in_maps.append({
            "x0T": x0, "x1T": x1,
            "wqk0": wqk0, "wqk1": wqk1, "bqk": bqk,
            "wv0": wv0, "wv1": wv1, "bv": bvB,
            "cosT": cosT, "sinT": sinT, "masks": masks,
            "ow0": ow0, "ow1": ow1, "ob": obB,
        })
    return in_maps


_NC_CACHE = {}


def kernel(x, rope_cos, rope_sin, qkv_w, qkv_b, out_w, out_b):
    B, T, D = x.shape
    H = 16
    NT = B * T
    S = NT // N_CORES
    key = (B, T, D, H)
    if key not in _NC_CACHE:
        _NC_CACHE[key] = build_nc(B, T, D, H)
    nc = _NC_CACHE[key]
    in_maps = _prep_inputs(
        np.asarray(x), np.asarray(rope_cos), np.asarray(rope_sin),
        np.asarray(qkv_w), np.asarray(qkv_b), np.asarray(out_w),
        np.asarray(out_b), B, T, D, H,
    )
    res = run_bass_kernel_spmd(nc, in_maps, core_ids=list(range(N_CORES)))
    out = np.empty((NT, D), np.float32)
    for c in range(N_CORES):
        out[c * S:(c + 1) * S] = np.asarray(res.results[c]["out"]).astype(np.float32)
    return out.reshape(B, T, D)
